# revision 2
# baseline (speedup 1.0000x reference)
"""BiMamba Trainium2 Bass kernel v2.

Sharding: data-parallel over batch - 8 NeuronCores, one batch element each.
Each core runs both directional Mamba blocks (fwd on x, bwd on host-flipped
x) in channel-major layout (d on partitions, L free), full-L [128,2048]
tiles.

Engine budget (per core, both directions):
- DVE runs ONLY the 128 tensor_tensor_scan ops (scan is DVE-only and has
  no fast mode: ~2.15 ns/elem -> ~565 us; everything else must stay off
  DVE).
- Pool (gpsimd) runs the elementwise muls as plain tensor_tensor (dbx,
  hc, u, y-accumulate for odd dt tiles, y2).
- ACT computes da_n = exp(-n*delta) directly (scale=-n, fp16 out), plus
  silu/softplus(exp+ln)/copies, biases via per-partition bias APs.
- PE does projections plus y-accumulation for even dt tiles via identity
  matmuls into a persistent 4-bank PSUM tile.
- Scan loop is n-outer over dt-pairs so each B_n/C_n broadcast DMA is
  shared by two dt tiles.
"""

import os
from contextlib import ExitStack

import ml_dtypes
import numpy as np

import concourse.bacc as bacc
import concourse.bass as bass
import concourse.mybir as mybir
import concourse.tile as tile

F32 = mybir.dt.float32
BF16 = mybir.dt.bfloat16
FP16 = mybir.dt.float16
AF = mybir.ActivationFunctionType
MUL = mybir.AluOpType.mult
ADD = mybir.AluOpType.add

D_MODEL = 256
N_STATE = 16
D_INNER = 512
DT_RANK = 16
D_CONV = 4
B_SZ, L = 8, 2048
NDT = 4
NCH = 4
NET = 2

SIM_COMPAT = bool(int(os.environ.get("BIMAMBA_SIM", "0")))
SCB = 2 if SIM_COMPAT else 3          # scan-pool depth (smaller in sim)

bf = ml_dtypes.bfloat16

_CACHE = {}
_LAST = {}


class _Ctx:
    def __init__(self, nc):
        self.nc = nc
        self.qi = 0

    def qeng(self):
        self.qi += 1
        return self.nc.sync


def _silu(nc, yp, dst, src_psum, bias):
    if SIM_COMPAT:
        tmp = yp.tile(list(dst.shape), F32, name=f"sb_{nc.next_id()}",
                      tag="sgb", bufs=2)
        nc.scalar.activation(tmp, src_psum, AF.Identity, bias=bias)
        sg = yp.tile(list(dst.shape), F32, name=f"sg_{nc.next_id()}",
                     tag="sg", bufs=2)
        nc.scalar.activation(sg, tmp, AF.Sigmoid)
        nc.vector.tensor_tensor(dst, sg, tmp, MUL)
    else:
        nc.scalar.activation(dst, src_psum, AF.Silu, bias=bias)


def _build_nc():
    nc = bacc.Bacc("TRN2", target_bir_lowering=False, debug=False)

    xp_d = nc.dram_tensor("xp", [2, D_MODEL, L + 3], BF16,
                          kind="ExternalInput")
    w2_d = nc.dram_tensor("w2", [2, 8, 128, D_INNER], BF16,
                          kind="ExternalInput")
    wz_d = nc.dram_tensor("wz", [2, 2, 128, D_INNER], BF16,
                          kind="ExternalInput")
    wxp_d = nc.dram_tensor("wxp", [2, 4, 128, 48], BF16, kind="ExternalInput")
    wdt_d = nc.dram_tensor("wdt", [2, 4, 16, 128], BF16, kind="ExternalInput")
    wo_d = nc.dram_tensor("wo", [2, 4, 128, D_MODEL], BF16,
                          kind="ExternalInput")
    bxc_d = nc.dram_tensor("bxc", [2, 128, 4], F32, kind="ExternalInput")
    bdt_d = nc.dram_tensor("bdt", [2, 128, 4], F32, kind="ExternalInput")
    dvec_d = nc.dram_tensor("dvec", [2, 128, 4], F32, kind="ExternalInput")
    id_d = nc.dram_tensor("ident", [128, 128], BF16, kind="ExternalInput")
    out_d = nc.dram_tensor("out", [2, D_MODEL, L], BF16, kind="ExternalOutput")

    cx = _Ctx(nc)

    with tile.TileContext(nc) as tc, ExitStack() as ctx:
        wpool = ctx.enter_context(tc.tile_pool(name="wpool", bufs=2))
        const = ctx.enter_context(tc.tile_pool(name="const", bufs=1))
        big = ctx.enter_context(tc.tile_pool(name="big", bufs=1))
        scanp = ctx.enter_context(tc.tile_pool(name="scanp", bufs=2))
        yp = ctx.enter_context(tc.tile_pool(name="yp", bufs=2))
        psum = ctx.enter_context(tc.tile_pool(name="psum", bufs=2,
                                              space="PSUM"))
        ypsum = ctx.enter_context(tc.tile_pool(name="ypsum", bufs=1,
                                               space="PSUM"))
        dram = ctx.enter_context(tc.tile_pool(name="dram", bufs=1,
                                              space="DRAM"))
        pools = dict(wpool=wpool, const=const, big=big, scanp=scanp, yp=yp,
                     psum=psum, ypsum=ypsum, dram=dram)
        tens = dict(xp_d=xp_d, w2_d=w2_d, wz_d=wz_d, wxp_d=wxp_d,
                    wdt_d=wdt_d, wo_d=wo_d, bxc_d=bxc_d, bdt_d=bdt_d,
                    dvec_d=dvec_d, id_d=id_d, out_d=out_d)

        ident_sb = const.tile([128, 128], BF16, name="ident")
        nc.sync.dma_start(ident_sb, id_d[:, :])
        tens["ident_sb"] = ident_sb

        st0 = _phase_a(nc, cx, 0, pools, tens, full=True)
        _phase_round(nc, cx, 0, st0, pools, tens, (0, 1))
        st1 = _phase_a(nc, cx, 1, pools, tens, full=False)
        _phase_round(nc, cx, 0, st0, pools, tens, (2, 3))
        _phase_tail(nc, cx, 0, st0, pools, tens)
        _phase_a2(nc, cx, 1, st1, pools, tens)
        _phase_round(nc, cx, 1, st1, pools, tens, (0, 1))
        _phase_round(nc, cx, 1, st1, pools, tens, (2, 3))
        _phase_tail(nc, cx, 1, st1, pools, tens)

    nc.compile()
    return nc


def _phase_a(nc, cx, di, pools, tens, full):
    """x load; xc = silu(W2@x + conv_b); xdbl = xproj@xc (+DRAM staging of
    B/C rows); delta = softplus via exp+ln. If full: also z, u, y0."""
    wpool, const, big, yp = (pools[k] for k in ("wpool", "const", "big", "yp"))
    psum, dram = pools["psum"], pools["dram"]
    xp_d, w2_d, wxp_d, wdt_d = (tens[k] for k in
                                ("xp_d", "w2_d", "wxp_d", "wdt_d"))

    st = dict(di=di, u={}, y={}, y2={}, sz={}, xc={}, delta={})

    x_sb = []
    for t2 in range(NET):
        t = big.tile([128, L + 3], BF16, name=f"x_{di}_{t2}", tag=f"x{t2}",
                     bufs=1)
        nc.sync.dma_start(t, xp_d[di, t2 * 128:(t2 + 1) * 128, :])
        x_sb.append(t)
    st["x_sb"] = x_sb

    bias_sb = const.tile([128, 12], F32, name=f"bias_{di}")
    nc.sync.dma_start(bias_sb[:, 0:4], tens["bxc_d"][di, :, :])
    nc.sync.dma_start(bias_sb[:, 4:8], tens["bdt_d"][di, :, :])
    nc.sync.dma_start(bias_sb[:, 8:12], tens["dvec_d"][di, :, :])
    st["bias"] = bias_sb

    # xc = silu(W2 @ x + conv_b)
    for et in range(NDT):
        w2_et = []
        for ks in range(8):
            w = wpool.tile([128, 128], BF16, name=f"w2_{di}_{et}_{ks}",
                           tag="wk", bufs=16)
            nc.sync.dma_start(w, w2_d[di, ks, :, et * 128:(et + 1) * 128])
            w2_et.append(w)
        xct = big.tile([128, L], BF16, name=f"xc_{di}_{et}", tag=f"xc{et}",
                       bufs=1)
        st["xc"][et] = xct
        for ch in range(NCH):
            pt = psum.tile([128, 512], F32, name=f"pxc_{di}_{et}_{ch}",
                           tag="mm", bufs=4)
            for ks in range(8):
                k, t2 = ks // NET, ks % NET
                rhs = x_sb[t2][:, k + ch * 512: k + ch * 512 + 512]
                nc.tensor.matmul(pt, w2_et[ks], rhs, start=(ks == 0),
                                 stop=(ks == 7))
            _silu(nc, yp, xct[:, ch * 512:(ch + 1) * 512], pt,
                  bias_sb[:, et:et + 1])

    if full:
        _phase_z(nc, cx, di, st, pools, tens)

    # xdbl = xproj_w @ xc  (rows 0:16 dt, 16:32 B, 32:48 C)
    wxp_sb = []
    for ks in range(NDT):
        w = wpool.tile([128, 48], BF16, name=f"wxp_{di}_{ks}", tag="wxp",
                       bufs=4)
        nc.sync.dma_start(w, wxp_d[di, ks, :, :])
        wxp_sb.append(w)
    xdbl = big.tile([48, L], BF16, name=f"xdbl_{di}", tag="xdbl", bufs=2)
    for ch in range(NCH):
        pt = psum.tile([48, 512], F32, name=f"pxp_{di}_{ch}", tag="mm",
                       bufs=4)
        for ks in range(NDT):
            nc.tensor.matmul(pt, wxp_sb[ks],
                             st["xc"][ks][:, ch * 512:(ch + 1) * 512],
                             start=(ks == 0), stop=(ks == NDT - 1))
        nc.scalar.copy(xdbl[:, ch * 512:(ch + 1) * 512], pt)
    st["xdbl"] = xdbl

    bcst = dram.tile([32, L], BF16, name=f"bcst_{di}", tag=f"bcst{di}")
    nc.sync.dma_start(bcst, xdbl[16:48, :])
    st["bcst"] = bcst

    # delta = ln(1 + exp(wdt @ xdbl_dt + dt_b))
    wdt_sb = []
    for mt in range(NDT):
        w = wpool.tile([16, 128], BF16, name=f"wdt_{di}_{mt}", tag="wdt",
                       bufs=8)
        nc.sync.dma_start(w, wdt_d[di, mt, :, :])
        wdt_sb.append(w)
    for mt in range(NDT):
        det = big.tile([128, L], BF16, name=f"de_{di}_{mt}", tag=f"de{di}{mt}",
                       bufs=1)
        st["delta"][mt] = det
        tmps = []
        for ch in range(NCH):
            pt = psum.tile([128, 512], F32, name=f"pde_{di}_{mt}_{ch}",
                           tag="mm", bufs=4)
            nc.tensor.matmul(pt, wdt_sb[mt],
                             xdbl[0:16, ch * 512:(ch + 1) * 512],
                             start=True, stop=True)
            tmp = yp.tile([128, 512], BF16, name=f"sp_{di}_{mt}_{ch}",
                          tag="sp", bufs=4)
            nc.scalar.activation(tmp, pt, AF.Exp,
                                 bias=bias_sb[:, 4 + mt:5 + mt])
            tmps.append(tmp)
        for ch in range(NCH):
            nc.scalar.activation(det[:, ch * 512:(ch + 1) * 512], tmps[ch],
                                 AF.Ln, bias=1.0)

    if full:
        _phase_uy(nc, cx, di, st, pools)
    return st


def _phase_z(nc, cx, di, st, pools, tens):
    wpool, big, psum = pools["wpool"], pools["big"], pools["psum"]
    x_sb = st["x_sb"]
    wz_sb = []
    for ks in range(NET):
        w = wpool.tile([128, D_INNER], BF16, name=f"wz_{di}_{ks}", tag="wz",
                       bufs=2)
        nc.sync.dma_start(w, tens["wz_d"][di, ks, :, :])
        wz_sb.append(w)
    for dt in range(NDT):
        zst = big.tile([128, L], BF16, name=f"zs_{di}_{dt}",
                       tag=f"zs{dt}", bufs=1)
        st["sz"][dt] = zst
        for ch in range(NCH):
            zp = psum.tile([128, 512], F32, name=f"pz_{di}_{dt}_{ch}",
                           tag="mm", bufs=4)
            for ks in range(NET):
                rhs = x_sb[ks][:, 3 + ch * 512: 3 + ch * 512 + 512]
                nc.tensor.matmul(zp, wz_sb[ks][:, dt * 128:(dt + 1) * 128],
                                 rhs, start=(ks == 0), stop=(ks == NET - 1))
            _silu(nc, pools["yp"], zst[:, ch * 512:(ch + 1) * 512], zp, 0.0)


def _phase_uy(nc, cx, di, st, pools):
    """u = delta*xc (Pool); dxc = D*xc via ACT copy-with-scale. For odd dts
    dxc doubles as the SBUF y accumulator seed; for even dts it is added
    at y2 time (PSUM holds the n-sum only)."""
    big = pools["big"]
    st["dxc"] = {}
    for dt in range(NDT):
        ut = big.tile([128, L], BF16, name=f"u_{di}_{dt}", tag=f"u{dt}",
                      bufs=1)
        nc.vector.tensor_tensor(ut, st["delta"][dt], st["xc"][dt], MUL)
        st["u"][dt] = ut
        tag = f"y{dt}" if dt % 2 == 1 else f"dxc{dt}"
        dxc = big.tile([128, L], BF16, name=f"dxc_{di}_{dt}", tag=tag,
                       bufs=1)
        nc.scalar.activation(dxc, st["xc"][dt], AF.Copy,
                             scale=st["bias"][:, 8 + dt:9 + dt])
        if dt % 2 == 1:
            st["y"][dt] = dxc
        else:
            st["dxc"][dt] = dxc


def _phase_a2(nc, cx, di, st, pools, tens):
    _phase_z(nc, cx, di, st, pools, tens)
    _phase_uy(nc, cx, di, st, pools)


def _phase_round(nc, cx, di, st, pools, tens, dts):
    """Scan round for dt pair (even, odd): n-outer; B_n/C_n broadcast once;
    per dt: da = exp(-n*delta) (ACT), dbx (Pool), scan (DVE), hc (Pool);
    even dt accumulates in PSUM via PE identity matmuls (start = ddiag@xc),
    odd dt accumulates in SBUF via Pool adds."""
    scanp, big, ypsum = pools["scanp"], pools["big"], pools["ypsum"]
    ident_sb = tens["ident_sb"]
    bcst = st["bcst"]
    H = L // 2
    ev = dts[0]

    yps = ypsum.tile([128, L], F32, name=f"yps_{di}_{ev}", tag="yps", bufs=1)

    for ni, n in enumerate(range(1, N_STATE + 1)):
        bb = scanp.tile([128, L], BF16, name=f"bb_{di}_{ev}_{n}", tag="bb",
                        bufs=2)
        cc = scanp.tile([128, L], BF16, name=f"cc_{di}_{ev}_{n}", tag="cc",
                        bufs=2)
        for half in range(2):
            for row, t in ((n - 1, bb), (16 + n - 1, cc)):
                src = bcst[row:row + 1, half * H:(half + 1) * H]
                cx.qeng().dma_start(
                    t[:, half * H:(half + 1) * H],
                    bass.AP(tensor=src.tensor, offset=src.offset,
                            ap=[[0, 128]] + list(src.ap[1:])))
        for dt in dts:
            da = scanp.tile([128, L], FP16, name=f"da_{di}_{dt}_{n}",
                            tag="da", bufs=SCB)
            nc.scalar.activation(da, st["delta"][dt], AF.Exp, scale=-float(n))
            dbx = scanp.tile([128, L], BF16, name=f"dbx_{di}_{dt}_{n}",
                             tag="dbx", bufs=SCB)
            nc.vector.tensor_tensor(dbx, st["u"][dt], bb, MUL)
            h = scanp.tile([128, L], BF16, name=f"h_{di}_{dt}_{n}",
                           tag="h", bufs=SCB)
            nc.vector.tensor_tensor_scan(h, da, dbx, 0.0, MUL, ADD)
            hc = scanp.tile([128, L], BF16, name=f"hc_{di}_{dt}_{n}",
                            tag="hc", bufs=SCB)
            nc.vector.tensor_tensor(hc, h, cc, MUL)
            if dt == ev:
                for ch in range(NCH):
                    nc.tensor.matmul(yps[:, ch * 512:(ch + 1) * 512],
                                     ident_sb,
                                     hc[:, ch * 512:(ch + 1) * 512],
                                     start=(n == 1), stop=(n == N_STATE))
            else:
                # y += hc on the DMA engines (software-DGE accumulate);
                # keeps both DVE and Pool free
                nc.gpsimd.dma_start(st["y"][dt], hc, accum_op=ADD)

    # y2 = (y [+ D*xc]) * silu(z)
    od = dts[1]
    y2e = big.tile([128, L], BF16, name=f"y2_{di}_{ev}", tag=f"y2{ev}",
                   bufs=1)
    for ch in range(NCH):
        sl = slice(ch * 512, (ch + 1) * 512)
        t = pools["yp"].tile([128, 512], BF16, name=f"yt_{di}_{ev}_{ch}",
                             tag="yt", bufs=1)
        nc.vector.tensor_tensor(t, yps[:, sl], st["dxc"][ev][:, sl], ADD)
        nc.vector.tensor_tensor(y2e[:, sl], t, st["sz"][ev][:, sl], MUL)
    st["y2"][ev] = y2e
    y2o = big.tile([128, L], BF16, name=f"y2_{di}_{od}", tag=f"y2{od}",
                   bufs=1)
    nc.vector.tensor_tensor(y2o, st["y"][od], st["sz"][od], MUL)
    st["y2"][od] = y2o


def _phase_tail(nc, cx, di, st, pools, tens):
    wpool, yp, psum = pools["wpool"], pools["yp"], pools["psum"]
    wo_d, out_d = tens["wo_d"], tens["out_d"]
    for ot in range(NET):
        wo_sb = []
        for ks in range(NDT):
            w = wpool.tile([128, 128], BF16, name=f"wo_{di}_{ot}_{ks}",
                           tag="wk", bufs=16)
            nc.sync.dma_start(w, wo_d[di, ks, :, ot * 128:(ot + 1) * 128])
            wo_sb.append(w)
        for ch in range(NCH):
            pt = psum.tile([128, 512], F32, name=f"po_{di}_{ot}_{ch}",
                           tag="mm", bufs=4)
            for ks in range(NDT):
                nc.tensor.matmul(pt, wo_sb[ks],
                                 st["y2"][ks][:, ch * 512:(ch + 1) * 512],
                                 start=(ks == 0), stop=(ks == NDT - 1))
            osb = yp.tile([128, 512], BF16, name=f"os_{di}_{ot}_{ch}",
                          tag="os", bufs=2)
            nc.scalar.copy(osb, pt)
            nc.sync.dma_start(
                out_d[di, ot * 128:(ot + 1) * 128,
                      ch * 512:(ch + 1) * 512], osb)


# ---------------------------------------------------------------------------
# host side
# ---------------------------------------------------------------------------

def _prep_dir(tw):
    in_w = tw["in_w"].astype(np.float64)        # (1024, 256)
    conv_w = tw["conv_w"].astype(np.float64)    # (512, 4)
    conv_b = tw["conv_b"].astype(np.float32)    # (512,)
    xproj = tw["xproj_w"].astype(np.float64)    # (48, 512)
    dt_w = tw["dt_w"].astype(np.float64)        # (512, 16)
    dt_b = tw["dt_b"].astype(np.float32)        # (512,)
    a_log = tw["A_log"].astype(np.float64)      # (512, 16)
    dvec = tw["D"].astype(np.float32)           # (512,)
    out_w = tw["out_w"].astype(np.float64)      # (256, 512)

    a_mat = -np.exp(a_log)
    assert np.allclose(a_mat, np.broadcast_to(
        -np.arange(1, N_STATE + 1, dtype=np.float64), a_mat.shape),
        rtol=1e-4, atol=1e-4), \
        "A_log is not log(arange(1..n_state+1)); exp(-n*delta) path invalid"

    win_xi = in_w[:D_INNER]
    win_z = in_w[D_INNER:]

    w2 = np.zeros((8, 128, D_INNER), np.float64)
    for k in range(D_CONV):
        for t2 in range(NET):
            w2[k * NET + t2] = (win_xi[:, t2 * 128:(t2 + 1) * 128].T
                                * conv_w[:, k][None, :])
    wz = np.stack([win_z[:, i * 128:(i + 1) * 128].T for i in range(NET)])
    wxp = np.stack([xproj.T[i * 128:(i + 1) * 128] for i in range(NDT)])
    wdt = np.stack([dt_w.T[:, i * 128:(i + 1) * 128] for i in range(NDT)])
    wo = np.stack([out_w.T[i * 128:(i + 1) * 128] for i in range(NDT)])

    bxc = conv_b.reshape(NDT, 128).T
    bdt = dt_b.reshape(NDT, 128).T
    dv = dvec.reshape(NDT, 128).T
    return dict(w2=w2, wz=wz, wxp=wxp, wdt=wdt, wo=wo, bxc=bxc, bdt=bdt,
                dvec=dv)


def kernel(**inputs):
    x = np.asarray(inputs["x"], np.float32)     # (8, 256, 2048)

    prep = []
    for tag in ("fwd", "bwd"):
        tw = {k[len(tag) + 1:]: np.asarray(v) for k, v in inputs.items()
              if k.startswith(tag + "_")}
        prep.append(_prep_dir(tw))

    if ("nc",) not in _CACHE:
        _CACHE[("nc",)] = _build_nc()
    nc = _CACHE[("nc",)]

    def stk(key, dtype):
        return np.ascontiguousarray(
            np.stack([np.asarray(p[key]) for p in prep]).astype(dtype))

    common = dict(
        w2=stk("w2", bf), wz=stk("wz", bf), wxp=stk("wxp", bf),
        wdt=stk("wdt", bf), wo=stk("wo", bf),
        bxc=stk("bxc", np.float32), bdt=stk("bdt", np.float32),
        dvec=stk("dvec", np.float32), ident=np.eye(128, dtype=bf),
    )

    in_maps = []
    for b in range(B_SZ):
        xp = np.zeros((2, D_MODEL, L + 3), bf)
        xp[0, :, 3:] = x[b].astype(bf)
        xp[1, :, 3:] = x[b, :, ::-1].astype(bf)
        in_maps.append(dict(common, xp=xp))

    _LAST["in_maps"] = in_maps

    if SIM_COMPAT:
        from concourse.bass_interp import CoreSim
        nb = int(os.environ.get("BIMAMBA_SIM_NB", "1"))
        res = []
        for b_i in range(nb):
            sim = CoreSim(nc, trace=False)
            for k, v in in_maps[b_i].items():
                sim.tensor(k)[:] = v
            sim.simulate()
            res.append(dict(out=np.array(sim.tensor("out"))))
        while len(res) < B_SZ:
            res.append(res[-1])
    else:
        from concourse.bass_utils import run_bass_kernel_spmd
        r = run_bass_kernel_spmd(nc, in_maps, core_ids=list(range(B_SZ)))
        res = r.results

    out = np.empty((B_SZ, 2 * D_MODEL, L), np.float32)
    for b in range(B_SZ):
        o = np.asarray(res[b]["out"], dtype=np.float32)
        out[b, :D_MODEL] = o[0]
        out[b, D_MODEL:] = o[1][:, ::-1]
    return out
